# revision 8
# baseline (speedup 1.0000x reference)
"""Multi-head attention (RoPE, causal) Trainium2 Bass kernel, 8-core SPMD.

Sharding: tensor-parallel over heads (2 heads/core) for QKV+attention,
AllToAll to token-shard for the output projection, host concat of row shards.

Math per core c (heads h0=2c, h1=2c+1), all matmuls fp32r:
  qT = Wq2.T @ xT            (feature-on-partition layout throughout)
  rope: qrot = qT*cosT + P2@(qT*sinT)   (P2 = pair-swap-with-sign, const)
  scoresT[k,q] = krot.T @ qrot tiles    ([k-part, q-free] layout)
  attnT = exp(scoresT/8), causal-masked on diagonal tiles
  AV: out[65,q] = [v | ones].T @ attnT  (row 64 = softmax denominator)
  normalize, AllToAll -> attn_catT [1024, 512tok], out = attn_catT.T @ Wo
"""
import numpy as np
from contextlib import ExitStack

import concourse.bass as bass
import concourse.mybir as mybir
import concourse.tile as tile
from concourse.bass_utils import run_bass_kernel_spmd

N_CORES = 8
B, S, D, H, DK = 2, 2048, 1024, 16, 64
T = B * S                    # 4096 flat tokens, batch-major
TT = 512                     # token tile (phase 1 / q tiles)
KT = 128                     # k tile (scores partition dim)
NT = T // TT                 # 8 token tiles
F32 = mybir.dt.float32
F32R = mybir.dt.float32r
AF = mybir.ActivationFunctionType
SCALE = 1.0 / np.sqrt(DK)

_cache = {}


def _consts():
    inv_freq = 10000.0 ** (-(np.arange(0, DK, 2, dtype=np.float64) / DK))
    pos = np.arange(S, dtype=np.float64)
    ang = pos[:, None] * inv_freq[None, :]                 # [S, 32]
    cos = np.repeat(np.cos(ang), 2, axis=1).T              # [64, S]
    sin = np.repeat(np.sin(ang), 2, axis=1).T
    cosT = np.concatenate([cos, cos], 0).astype(np.float32)   # [128, S]
    sinT = np.concatenate([sin, sin], 0).astype(np.float32)
    # P2T = P.T blockdiag for 2 heads; (P v)[2i] = -v[2i+1], (P v)[2i+1] = v[2i]
    p = np.zeros((DK, DK), np.float32)
    for i in range(DK // 2):
        p[2 * i, 2 * i + 1] = -1.0
        p[2 * i + 1, 2 * i] = 1.0
    p2t = np.zeros((128, 128), np.float32)
    p2t[:DK, :DK] = p.T
    p2t[DK:, DK:] = p.T
    ident = np.eye(128, dtype=np.float32)
    ones64 = np.ones((1, DK), np.float32)
    return cosT, sinT, p2t, ident, ones64


def split_multi_waits(nc, max_waits=1):
    """This walrus build allows fewer sync-waits per instruction than Tile's
    final drain carries; hoist extras onto same-engine NOPs inserted before."""
    for fn in nc.m.functions:
        for blk in fn.blocks:
            insts = blk.instructions
            out = []
            for inst in insts:
                si = getattr(inst, "sync_info", None)
                waits = list(si.on_wait) if si is not None else []
                if len(waits) > max_waits:
                    extra, keep = waits[:-max_waits], waits[-max_waits:]
                    for j, w in enumerate(extra):
                        nop = mybir.InstNoOp(
                            name=f"{inst.name}-wsplit{j}", ins=[], outs=[]
                        )
                        nop.engine = inst.engine
                        nop.sync_info = mybir.SyncInfo(on_wait=[w], on_update=[])
                        out.append(nop)
                    inst.sync_info = mybir.SyncInfo(
                        on_wait=keep, on_update=list(si.on_update)
                    )
                out.append(inst)
            insts[:] = out


def build_nc():
    cosT_np, sinT_np, p2t_np, ident_np, ones64_np = _consts()

    nc = bass.Bass("TRN2", target_bir_lowering=False, debug=False,
                   num_devices=N_CORES)
    xT = nc.declare_dram_parameter("xT", [D, T], F32R, isOutput=False)
    wq = nc.declare_dram_parameter("wq", [D, 128], F32R, isOutput=False)
    wk = nc.declare_dram_parameter("wk", [D, 128], F32R, isOutput=False)
    wv = nc.declare_dram_parameter("wv", [D, 128], F32R, isOutput=False)
    wo = nc.declare_dram_parameter("wo", [D, D], F32R, isOutput=False)
    y = nc.declare_dram_parameter("y", [TT, D], F32, isOutput=True)

    c_cos = nc.inline_tensor(cosT_np, name="c_cos")
    c_sin = nc.inline_tensor(sinT_np, name="c_sin")
    c_p2t = nc.inline_tensor(p2t_np, name="c_p2t")
    c_id = nc.inline_tensor(ident_np, name="c_id")
    c_on = nc.inline_tensor(ones64_np, name="c_on")

    a2a_in = nc.dram_tensor("a2a_in", [N_CORES, 128, TT], F32R)
    a2a_out = nc.dram_tensor("a2a_out", [N_CORES, 128, TT], F32R)

    with tile.TileContext(nc) as tc, ExitStack() as ctx:
        cst = ctx.enter_context(tc.tile_pool(name="cst", bufs=1))
        stream = ctx.enter_context(tc.tile_pool(name="stream", bufs=2))
        persist = ctx.enter_context(tc.tile_pool(name="persist", bufs=1))
        tmp = ctx.enter_context(tc.tile_pool(name="tmp", bufs=3))
        attnp = ctx.enter_context(tc.tile_pool(name="attnp", bufs=4))
        outp = ctx.enter_context(tc.tile_pool(name="outp", bufs=3))
        ps = ctx.enter_context(tc.tile_pool(name="ps", bufs=4, space="PSUM"))
        psav = ctx.enter_context(tc.tile_pool(name="psav", bufs=3, space="PSUM"))

        # ---- constants + weights to SBUF ----
        cos_s = cst.tile([128, S], F32)
        sin_s = cst.tile([128, S], F32)
        p2t_s = cst.tile([128, 128], F32R)
        id_s = cst.tile([128, 128], F32R)
        on_s = cst.tile([1, DK], F32R)
        nc.sync.dma_start(out=cos_s[:], in_=c_cos[:, :])
        nc.sync.dma_start(out=sin_s[:], in_=c_sin[:, :])
        nc.sync.dma_start(out=p2t_s[:], in_=c_p2t.ap().bitcast(F32R))
        nc.sync.dma_start(out=id_s[:], in_=c_id.ap().bitcast(F32R))
        nc.sync.dma_start(out=on_s[:], in_=c_on.ap().bitcast(F32R))

        wq_s = cst.tile([128, 8, 128], F32R)
        wk_s = cst.tile([128, 8, 128], F32R)
        wv_s = cst.tile([128, 8, 128], F32R)
        for w_dram, w_sb in ((wq, wq_s), (wk, wk_s), (wv, wv_s)):
            nc.sync.dma_start(
                out=w_sb[:], in_=w_dram.ap().rearrange("(g p) m -> p g m", p=128)
            )

        # persistent activations
        qrot = persist.tile([128, T], F32R)
        krot = persist.tile([128, T], F32R)
        v_sb = persist.tile([128, T // KT, 130], F32R)   # [.., 0:64]+ones | [.., 65:129]+ones
        # cols 64 and 129 stay 1.0 (denominator ones); memset needs an f32 view
        nc.vector.memset(v_sb[:].rearrange("p a b -> p (a b)").bitcast(F32), 1.0)

        def phase1(t):
            """Project token tile t, rope q/k, transpose v."""
            xt = stream.tile([128, 8, TT], F32R, tag="xt")
            nc.sync.dma_start(
                out=xt[:],
                in_=xT[:, t * TT:(t + 1) * TT].rearrange("(g p) n -> p g n", p=128),
            )
            pos = slice((t % (S // TT)) * TT, (t % (S // TT)) * TT + TT)
            tok = slice(t * TT, (t + 1) * TT)
            for w_sb, dst in ((wq_s, qrot), (wk_s, krot)):
                p_q = ps.tile([128, TT], F32, tag="mm")
                for g in range(8):
                    nc.tensor.matmul(p_q[:], w_sb[:, g, :], xt[:, g, :],
                                     start=(g == 0), stop=(g == 7))
                qs = tmp.tile([128, TT], F32R, tag="qs")
                nc.vector.tensor_mul(qs[:], p_q[:], sin_s[:, pos])
                p_perm = ps.tile([128, TT], F32, tag="mm")
                nc.tensor.matmul(p_perm[:], p2t_s[:], qs[:], start=True, stop=True)
                qc = tmp.tile([128, TT], F32, tag="qc")
                nc.vector.tensor_mul(qc[:], p_q[:], cos_s[:, pos])
                nc.vector.tensor_add(dst[:, tok], qc[:], p_perm[:])
            # v: project then transpose to natural layout
            p_v = ps.tile([128, TT], F32, tag="mm")
            for g in range(8):
                nc.tensor.matmul(p_v[:], wv_s[:, g, :], xt[:, g, :],
                                 start=(g == 0), stop=(g == 7))
            vt = tmp.tile([128, TT], F32R, tag="vt")
            nc.vector.tensor_copy(vt[:], p_v[:])
            for blk in range(TT // 128):
                p_t = ps.tile([128, 128], F32R, tag="mm")
                nc.tensor.transpose(p_t[:], vt[:, blk * 128:(blk + 1) * 128], id_s[:])
                g = t * (TT // 128) + blk
                nc.vector.tensor_copy(v_sb[:, g, 0:64], p_t[:, 0:64])
                nc.vector.tensor_copy(v_sb[:, g, 65:129], p_t[:, 64:128])

        def attention(b, J):
            """q-tile J (512 wide) of batch b, both heads."""
            av0 = psav.tile([65, TT], F32, tag="av")
            av1 = psav.tile([65, TT], F32, tag="av")
            av = [av0, av1]
            nk = 4 * (J + 1)
            for i in range(nk):
                for h in range(2):
                    hp = slice(64 * h, 64 * h + 64)
                    p_s = ps.tile([128, TT], F32, tag="mm")
                    nc.tensor.matmul(
                        p_s[:],
                        krot[hp, b * S + i * KT: b * S + (i + 1) * KT],
                        qrot[hp, b * S + J * TT: b * S + (J + 1) * TT],
                        start=True, stop=True,
                    )
                    at = attnp.tile([128, TT], F32R, tag="at")
                    nc.scalar.activation(at[:], p_s[:], AF.Exp, scale=float(SCALE))
                    if i >= 4 * J:  # diagonal block: zero where k > q
                        nc.gpsimd.affine_select(
                            out=at[:], in_=at[:], compare_op=mybir.AluOpType.is_ge,
                            fill=0.0, base=J * TT - i * KT,
                            pattern=[[1, TT]], channel_multiplier=-1,
                        )
                    g = (b * S) // KT + i
                    nc.tensor.matmul(
                        av[h][:], v_sb[:, g, 65 * h:65 * h + 65], at[:],
                        start=(i == 0), stop=(i == nk - 1),
                    )
            # normalize by denominator row and stage for A2A
            stage = outp.tile([128, TT], F32R, tag="stage")
            for h in range(2):
                rec = tmp.tile([1, TT], F32R, tag="rec")
                with nc.allow_low_precision(reason="f32r recip for bcast matmul"):
                    nc.vector.reciprocal(rec[:], av[h][64:65, :])
                p_bc = ps.tile([64, TT], F32, tag="mm")
                nc.tensor.matmul(p_bc[:], on_s[:], rec[:], start=True, stop=True)
                hp = slice(64 * h, 64 * h + 64)
                nc.vector.tensor_copy(stage[hp, :], av[h][0:64, :])
                nc.vector.tensor_mul(stage[hp, :], stage[hp, :], p_bc[:])
            dest = 4 * b + J
            nc.sync.dma_start(out=a2a_in[dest, :, :], in_=stage[:])

        for t in range(4):
            phase1(t)
        for J in range(4):
            attention(0, J)
        for t in range(4, 8):
            phase1(t)
        for J in range(4):
            attention(1, J)

        nc.gpsimd.collective_compute(
            "AllToAll", mybir.AluOpType.bypass,
            replica_groups=[list(range(N_CORES))],
            ins=[a2a_in.ap().opt()], outs=[a2a_out.ap().opt()],
        )

        # ---- output projection on my 512-token row shard ----
        cat = persist.tile([128, 8, TT], F32R)
        nc.sync.dma_start(out=cat[:], in_=a2a_out.ap().rearrange("g p f -> p g f"))
        wo_s = stream.tile([128, 8, TT], F32R, tag="wo")
        for n in range(2):
            nc.sync.dma_start(
                out=wo_s[:],
                in_=wo[:, n * TT:(n + 1) * TT].rearrange("(g p) n -> p g n", p=128),
            )
            for m in range(4):
                po = ps.tile([128, TT], F32, tag="mm")
                for g in range(8):
                    nc.tensor.matmul(po[:], cat[:, g, m * 128:(m + 1) * 128],
                                     wo_s[:, g, :], start=(g == 0), stop=(g == 7))
                yt = outp.tile([128, TT], F32, tag="yt")
                nc.vector.tensor_copy(yt[:], po[:])
                nc.sync.dma_start(
                    out=y[m * 128:(m + 1) * 128, n * TT:(n + 1) * TT], in_=yt[:]
                )

    split_multi_waits(nc)
    return nc


def kernel(x, Wq, Wk, Wv, Wo):
    x, Wq, Wk, Wv, Wo = (np.asarray(a, np.float32) for a in (x, Wq, Wk, Wv, Wo))
    if "nc" not in _cache:
        _cache["nc"] = build_nc()
    nc = _cache["nc"]

    xT = np.ascontiguousarray(x.reshape(T, D).T)
    wo = np.ascontiguousarray(Wo)
    in_maps = []
    for c in range(N_CORES):
        in_maps.append({
            "xT": xT,
            "wq": np.ascontiguousarray(np.concatenate([Wq[2 * c], Wq[2 * c + 1]], 1)),
            "wk": np.ascontiguousarray(np.concatenate([Wk[2 * c], Wk[2 * c + 1]], 1)),
            "wv": np.ascontiguousarray(np.concatenate([Wv[2 * c], Wv[2 * c + 1]], 1)),
            "wo": wo,
        })
    res = run_bass_kernel_spmd(nc, in_maps, core_ids=list(range(N_CORES)))
    out = np.concatenate([r["y"] for r in res.results], axis=0)
    return out.reshape(B, S, D)


# revision 19
# speedup vs baseline: 275.4795x; 275.4795x over previous
"""Multi-head attention (RoPE, causal) Trainium2 Bass kernel, 8-core SPMD.

Sharding: tensor-parallel over heads (2 heads/core) for QKV+attention,
AllToAll to token-shard for the output projection, host concat of row shards.

Math per core c (heads h0=2c, h1=2c+1), all matmuls fp32r:
  qT = Wq2.T @ xT            (feature-on-partition layout throughout)
  rope: qrot = qT*cosT + P2@(qT*sinT)   (P2 = pair-swap-with-sign, const)
  scoresT[k,q] = krot.T @ qrot tiles    ([k-part, q-free] layout)
  attnT = exp(scoresT/8), causal-masked on diagonal tiles
  AV: out[65,q] = [v | ones].T @ attnT  (row 64 = softmax denominator)
  normalize, AllToAll -> attn_catT [1024, 512tok], out = attn_catT.T @ Wo
"""
import numpy as np
from contextlib import ExitStack

import concourse.bass as bass
import concourse.mybir as mybir
import concourse.tile as tile
from concourse.bass_utils import run_bass_kernel_spmd

N_CORES = 8
B, S, D, H, DK = 2, 2048, 1024, 16, 64
T = B * S                    # 4096 flat tokens, batch-major
TT = 512                     # token tile (phase 1 / q tiles)
KT = 128                     # k tile (scores partition dim)
NT = T // TT                 # 8 token tiles
F32 = mybir.dt.float32
F32R = mybir.dt.float32r
AF = mybir.ActivationFunctionType
SCALE = 1.0 / np.sqrt(DK)

_cache = {}


def _consts():
    inv_freq = 10000.0 ** (-(np.arange(0, DK, 2, dtype=np.float64) / DK))
    pos = np.arange(S, dtype=np.float64)
    ang = pos[:, None] * inv_freq[None, :]                 # [S, 32]
    cos = np.repeat(np.cos(ang), 2, axis=1).T              # [64, S]
    sin = np.repeat(np.sin(ang), 2, axis=1).T
    cosT = np.concatenate([cos, cos], 0).astype(np.float32)   # [128, S]
    sinT = np.concatenate([sin, sin], 0).astype(np.float32)
    # P2T = P.T blockdiag for 2 heads; (P v)[2i] = -v[2i+1], (P v)[2i+1] = v[2i]
    p = np.zeros((DK, DK), np.float32)
    for i in range(DK // 2):
        p[2 * i, 2 * i + 1] = -1.0
        p[2 * i + 1, 2 * i] = 1.0
    p2t = np.zeros((128, 128), np.float32)
    p2t[:DK, :DK] = p.T
    p2t[DK:, DK:] = p.T
    ident = np.eye(128, dtype=np.float32)
    ones64 = np.ones((1, DK), np.float32)
    return cosT, sinT, p2t, ident, ones64


def split_multi_waits(nc, max_waits=1):
    """This walrus build allows fewer sync-waits per instruction than Tile's
    final drain carries; hoist extras onto same-engine NOPs inserted before."""
    for fn in nc.m.functions:
        for blk in fn.blocks:
            insts = blk.instructions
            out = []
            for inst in insts:
                si = getattr(inst, "sync_info", None)
                waits = list(si.on_wait) if si is not None else []
                if len(waits) > max_waits:
                    extra, keep = waits[:-max_waits], waits[-max_waits:]
                    for j, w in enumerate(extra):
                        nop = mybir.InstNoOp(
                            name=f"{inst.name}-wsplit{j}", ins=[], outs=[]
                        )
                        nop.engine = inst.engine
                        nop.sync_info = mybir.SyncInfo(on_wait=[w], on_update=[])
                        out.append(nop)
                    inst.sync_info = mybir.SyncInfo(
                        on_wait=keep, on_update=list(si.on_update)
                    )
                out.append(inst)
            insts[:] = out


def build_nc(repeat=1):
    cosT_np, sinT_np, p2t_np, ident_np, ones64_np = _consts()

    nc = bass.Bass("TRN2", target_bir_lowering=False, debug=False,
                   num_devices=N_CORES)
    xT = nc.declare_dram_parameter("xT", [D, T], F32R, isOutput=False)
    wq = nc.declare_dram_parameter("wq", [D, 128], F32R, isOutput=False)
    wk = nc.declare_dram_parameter("wk", [D, 128], F32R, isOutput=False)
    wv = nc.declare_dram_parameter("wv", [D, 128], F32R, isOutput=False)
    wo = nc.declare_dram_parameter("wo", [D, D], F32R, isOutput=False)
    y = nc.declare_dram_parameter("y", [TT, D], F32, isOutput=True)

    c_cos = nc.inline_tensor(cosT_np, name="c_cos")
    c_sin = nc.inline_tensor(sinT_np, name="c_sin")
    c_p2t = nc.inline_tensor(p2t_np, name="c_p2t")
    c_id = nc.inline_tensor(ident_np, name="c_id")
    c_on = nc.inline_tensor(ones64_np, name="c_on")

    a2a_ins = [nc.dram_tensor(f"a2a_in{r}", [N_CORES, 128, TT], F32R)
               for r in range(repeat)]
    a2a_outs = [nc.dram_tensor(f"a2a_out{r}", [N_CORES, 128, TT], F32R)
                for r in range(repeat)]

    with tile.TileContext(nc) as tc, ExitStack() as ctx:
        cst = ctx.enter_context(tc.tile_pool(name="cst", bufs=1))
        stream = ctx.enter_context(tc.tile_pool(name="stream", bufs=2))
        persist = ctx.enter_context(tc.tile_pool(name="persist", bufs=1))
        tmp = ctx.enter_context(tc.tile_pool(name="tmp", bufs=3))
        attnp = ctx.enter_context(tc.tile_pool(name="attnp", bufs=3))
        outp = ctx.enter_context(tc.tile_pool(name="outp", bufs=3))
        ps = ctx.enter_context(tc.tile_pool(name="ps", bufs=2, space="PSUM"))
        psav = ctx.enter_context(tc.tile_pool(name="psav", bufs=2, space="PSUM"))

        # ---- constants + weights to SBUF ----
        cos_s = cst.tile([128, S], F32)
        sin_s = cst.tile([128, S], F32)
        p2t_s = cst.tile([128, 128], F32R)
        id_s = cst.tile([128, 128], F32R)
        on_s = cst.tile([1, DK], F32R)
        nc.gpsimd.dma_start(out=p2t_s[:], in_=c_p2t.ap().bitcast(F32R))
        nc.gpsimd.dma_start(out=id_s[:], in_=c_id.ap().bitcast(F32R))
        nc.gpsimd.dma_start(out=on_s[:], in_=c_on.ap().bitcast(F32R))

        wq_s = cst.tile([128, 8, 128], F32R)
        wk_s = cst.tile([128, 8, 128], F32R)
        wv_s = cst.tile([128, 8, 128], F32R)
        for w_dram, w_sb in ((wq, wq_s), (wk, wk_s), (wv, wv_s)):
            nc.gpsimd.dma_start(
                out=w_sb[:], in_=w_dram.ap().rearrange("(g p) m -> p g m", p=128)
            )
        nc.gpsimd.dma_start(out=cos_s[:], in_=c_cos[:, :])
        nc.gpsimd.dma_start(out=sin_s[:], in_=c_sin[:, :])

        # persistent activations
        qrot = persist.tile([128, T], F32R)
        krot = persist.tile([128, T], F32R)
        v_sb = persist.tile([128, T // KT, 130], F32R)   # [.., 0:64]+ones | [.., 65:129]+ones
        # cols 64 and 129 stay 1.0 (denominator ones); memset needs an f32 view
        nc.vector.memset(v_sb[:].rearrange("p a b -> p (a b)").bitcast(F32), 1.0)

        cur = {}

        def phase1(t):
            """Project token tile t, rope q/k, transpose v."""
            xt = stream.tile([128, 8, TT], F32R, tag="xt")
            for g in range(8):
                nc.sync.dma_start(
                    out=xt[:, g, :],
                    in_=xT[g * 128:(g + 1) * 128, t * TT:(t + 1) * TT],
                )
            pos = slice((t % (S // TT)) * TT, (t % (S // TT)) * TT + TT)
            tok = slice(t * TT, (t + 1) * TT)
            for w_sb, dst in ((wq_s, qrot), (wk_s, krot)):
                p_q = ps.tile([128, TT], F32, tag="p1")
                for g in range(8):
                    nc.tensor.matmul(p_q[:], w_sb[:, g, :], xt[:, g, :],
                                     start=(g == 0), stop=(g == 7))
                qs = tmp.tile([128, TT], F32R, tag="qs")
                nc.vector.tensor_mul(qs[:], p_q[:], sin_s[:, pos])
                p_perm = ps.tile([128, TT], F32, tag="p1")
                nc.tensor.matmul(p_perm[:], p2t_s[:], qs[:], start=True, stop=True)
                qc = tmp.tile([128, TT], F32, tag="qc")
                nc.vector.tensor_mul(qc[:], p_q[:], cos_s[:, pos])
                nc.vector.tensor_add(dst[:, tok], qc[:], p_perm[:])
            # v: project then transpose to natural layout
            p_v = ps.tile([128, TT], F32, tag="p1")
            for g in range(8):
                nc.tensor.matmul(p_v[:], wv_s[:, g, :], xt[:, g, :],
                                 start=(g == 0), stop=(g == 7))
            vt = tmp.tile([128, TT], F32R, tag="vt")
            nc.vector.tensor_copy(vt[:], p_v[:])
            for blk in range(TT // 128):
                p_t = ps.tile([128, 128], F32R, tag="p1")
                nc.tensor.transpose(p_t[:], vt[:, blk * 128:(blk + 1) * 128], id_s[:])
                g = t * (TT // 128) + blk
                nc.vector.tensor_copy(v_sb[:, g, 0:64], p_t[:, 0:64])
                nc.vector.tensor_copy(v_sb[:, g, 65:129], p_t[:, 64:128])

        def attention(b, J):
            """q-tile J (512 wide) of batch b, both heads paired."""
            av0 = psav.tile([65, TT], F32, tag="av")
            av1 = psav.tile([65, TT], F32, tag="av")
            av = [av0, av1]
            nk = 4 * (J + 1)
            for i in range(nk):
                r = i - 4 * J          # >= 0 on diagonal blocks
                qo = KT * r if r > 0 else 0    # causal-narrowed q offset
                n = TT - qo
                p_s = ps.tile([128, 2, TT], F32, tag="mm")
                for h in range(2):
                    hp = slice(64 * h, 64 * h + 64)
                    nc.tensor.matmul(
                        p_s[:, h, 0:n],
                        krot[hp, b * S + i * KT: b * S + (i + 1) * KT],
                        qrot[hp, b * S + J * TT + qo: b * S + (J + 1) * TT],
                        start=True, stop=True,
                    )
                at = attnp.tile([128, 2, TT], F32R, tag="at")
                nc.scalar.activation(at[:, :, 0:n], p_s[:, :, 0:n], AF.Exp,
                                     scale=float(SCALE))
                if r >= 0:  # diagonal 128-block: zero where k > q
                    for h in range(2):
                        nc.gpsimd.affine_select(
                            out=at[:, h, 0:KT], in_=at[:, h, 0:KT],
                            compare_op=mybir.AluOpType.is_ge,
                            fill=0.0, base=0,
                            pattern=[[1, KT]], channel_multiplier=-1,
                        )
                g = (b * S) // KT + i
                for h in range(2):
                    nc.tensor.matmul(
                        av[h][:, qo:TT], v_sb[:, g, 65 * h:65 * h + 65],
                        at[:, h, 0:n],
                        start=(i == 0), stop=(i == nk - 1),
                    )
            # normalize by denominator row and stage for A2A
            stage = outp.tile([128, TT], F32R, tag="stage")
            for h in range(2):
                rec = tmp.tile([1, TT], F32R, tag="rec")
                with nc.allow_low_precision(reason="f32r recip for bcast matmul"):
                    nc.vector.reciprocal(rec[:], av[h][64:65, :])
                p_bc = ps.tile([64, TT], F32, tag="p1")
                nc.tensor.matmul(p_bc[:], on_s[:], rec[:], start=True, stop=True)
                hp = slice(64 * h, 64 * h + 64)
                nc.vector.tensor_copy(stage[hp, :], av[h][0:64, :])
                nc.vector.tensor_mul(stage[hp, :], stage[hp, :], p_bc[:])
            dest = 4 * b + J
            nc.sync.dma_start(out=cur["a2a_in"][dest, :, :], in_=stage[:])

        # interleave: attention(b, J) depends only on token tiles <= t
        wo_s0 = cst.tile([128, 8, TT], F32R)
        wo_s1 = cst.tile([128, 8, TT], F32R)
        for rep in range(repeat):
            cur["a2a_in"] = a2a_ins[rep]
            for t in range(8):
                phase1(t)
                attention(t // 4, t % 4)
                if rep == 0 and t == 0:  # preload Wo while DMA is idle
                    for n, w_sb in ((0, wo_s0), (1, wo_s1)):
                        nc.gpsimd.dma_start(
                            out=w_sb[:],
                            in_=wo[:, n * TT:(n + 1) * TT].rearrange(
                                "(g p) n -> p g n", p=128),
                        )

            nc.gpsimd.collective_compute(
                "AllToAll", mybir.AluOpType.bypass,
                replica_groups=[list(range(N_CORES))],
                ins=[a2a_ins[rep].ap().opt()], outs=[a2a_outs[rep].ap().opt()],
            )

            # ---- output projection on my 512-token row shard ----
            cat = persist.tile([128, 8, TT], F32R, tag="cat")
            nc.sync.dma_start(out=cat[:],
                              in_=a2a_outs[rep].ap().rearrange("g p f -> p g f"))
            for n, wo_s in ((0, wo_s0), (1, wo_s1)):
                for m in range(4):
                    po = ps.tile([128, TT], F32, tag="p1")
                    for g in range(8):
                        nc.tensor.matmul(po[:], cat[:, g, m * 128:(m + 1) * 128],
                                         wo_s[:, g, :], start=(g == 0), stop=(g == 7))
                    yt = outp.tile([128, TT], F32, tag="yt")
                    nc.vector.tensor_copy(yt[:], po[:])
                    nc.sync.dma_start(
                        out=y[m * 128:(m + 1) * 128, n * TT:(n + 1) * TT], in_=yt[:]
                    )

    split_multi_waits(nc)
    return nc


def _get_runner(repeat=1):
    """Build + jit once; returns f(in_maps) -> list of per-core output dicts."""
    key = ("runner", repeat)
    if key in _cache:
        return _cache[key]
    import jax
    import jax.numpy as jnp
    from jax.sharding import Mesh, PartitionSpec
    from jax.experimental.shard_map import shard_map
    from concourse import bass2jax, mybir as _mybir

    nc = build_nc(repeat=repeat)
    bass2jax.install_neuronx_cc_hook()

    in_names, out_names, out_avals, zero_outs = [], [], [], []
    for alloc in nc.m.functions[0].allocations:
        if not isinstance(_mybir.MemoryLocationSet, type) or not isinstance(
            alloc, _mybir.MemoryLocationSet
        ):
            continue
        name = alloc.memorylocations[0].name
        if alloc.kind == "ExternalInput":
            if name != "partition_id":
                in_names.append(name)
        elif alloc.kind == "ExternalOutput":
            out_names.append(name)
            shape = tuple(alloc.tensor_shape)
            dtype = _mybir.dt.np(alloc.dtype)
            out_avals.append(jax.core.ShapedArray(shape, dtype))
            zero_outs.append(np.zeros(shape, dtype))
    n_params = len(in_names)
    has_pid = nc.partition_id_tensor is not None
    all_names = in_names + out_names + (["partition_id"] if has_pid else [])

    def _body(*args):
        operands = list(args)
        if has_pid:
            operands.append(bass2jax.partition_id_tensor())
        outs = bass2jax._bass_exec_p.bind(
            *operands,
            out_avals=tuple(out_avals),
            in_names=tuple(all_names),
            out_names=tuple(out_names),
            lowering_input_output_aliases=(),
            sim_require_finite=True,
            sim_require_nnan=True,
            nc=nc,
        )
        return tuple(outs)

    devices = jax.devices()[:N_CORES]
    mesh = Mesh(np.asarray(devices), ("core",))
    n_outs = len(out_names)
    sharded = jax.jit(
        shard_map(
            _body, mesh=mesh,
            in_specs=(PartitionSpec("core"),) * (n_params + n_outs),
            out_specs=(PartitionSpec("core"),) * n_outs,
            check_rep=False,
        ),
        donate_argnums=tuple(range(n_params, n_params + n_outs)),
        keep_unused=True,
    )

    def make_bench(in_maps):
        from jax.sharding import NamedSharding
        sh = NamedSharding(mesh, PartitionSpec("core"))
        concat_in = [
            jax.device_put(
                np.concatenate([np.asarray(m[nm]) for m in in_maps], axis=0), sh)
            for nm in in_names
        ]
        zshapes = [(N_CORES * z.shape[0], *z.shape[1:]) for z in zero_outs]
        zdt = [z.dtype for z in zero_outs]
        mkz = jax.jit(
            lambda: tuple(jnp.zeros(s, d) for s, d in zip(zshapes, zdt)),
            out_shardings=tuple(sh for _ in zshapes),
        )

        def bench_once():
            zs = mkz()
            jax.block_until_ready(zs)
            t0 = __import__("time").perf_counter()
            out = sharded(*concat_in, *zs)
            jax.block_until_ready(out)
            return __import__("time").perf_counter() - t0

        return bench_once

    run_ns = {"make_bench": make_bench}

    def run(in_maps):
        concat_in = [
            np.concatenate([np.asarray(m[nm]) for m in in_maps], axis=0)
            for nm in in_names
        ]
        concat_zeros = [
            np.zeros((N_CORES * z.shape[0], *z.shape[1:]), z.dtype)
            for z in zero_outs
        ]
        out_arrs = sharded(*concat_in, *concat_zeros)
        return [
            {nm: np.asarray(out_arrs[i]).reshape(N_CORES, *out_avals[i].shape)[c]
             for i, nm in enumerate(out_names)}
            for c in range(N_CORES)
        ]

    run.make_bench = make_bench
    _cache[key] = run
    return run


def _prep_in_maps(x, Wq, Wk, Wv, Wo):
    xT = np.ascontiguousarray(x.reshape(T, D).T)
    wo = np.ascontiguousarray(Wo)
    in_maps = []
    for c in range(N_CORES):
        in_maps.append({
            "xT": xT,
            "wq": np.ascontiguousarray(np.concatenate([Wq[2 * c], Wq[2 * c + 1]], 1)),
            "wk": np.ascontiguousarray(np.concatenate([Wk[2 * c], Wk[2 * c + 1]], 1)),
            "wv": np.ascontiguousarray(np.concatenate([Wv[2 * c], Wv[2 * c + 1]], 1)),
            "wo": wo,
        })
    return in_maps


def kernel(x, Wq, Wk, Wv, Wo, repeat=1):
    x, Wq, Wk, Wv, Wo = (np.asarray(a, np.float32) for a in (x, Wq, Wk, Wv, Wo))
    run = _get_runner(repeat=repeat)
    results = run(_prep_in_maps(x, Wq, Wk, Wv, Wo))
    out = np.concatenate([r["y"] for r in results], axis=0)
    return out.reshape(B, S, D)


# revision 21
# speedup vs baseline: 18812.1211x; 68.2886x over previous
"""Multi-head attention (RoPE, causal) Trainium2 Bass kernel, 8-core SPMD.

Sharding: tensor-parallel over heads (2 heads/core) for QKV+attention,
AllToAll to token-shard for the output projection, host concat of row shards.

Math per core c (heads h0=2c, h1=2c+1), all matmuls fp32r:
  qT = Wq2.T @ xT            (feature-on-partition layout throughout)
  rope: qrot = qT*cosT + P2@(qT*sinT)   (P2 = pair-swap-with-sign, const)
  scoresT[k,q] = krot.T @ qrot tiles    ([k-part, q-free] layout)
  attnT = exp(scoresT/8), causal-masked on diagonal tiles
  AV: out[65,q] = [v | ones].T @ attnT  (row 64 = softmax denominator)
  normalize, AllToAll -> attn_catT [1024, 512tok], out = attn_catT.T @ Wo
"""
import numpy as np
from contextlib import ExitStack

import concourse.bass as bass
import concourse.mybir as mybir
import concourse.tile as tile
from concourse.bass_utils import run_bass_kernel_spmd

N_CORES = 8
B, S, D, H, DK = 2, 2048, 1024, 16, 64
T = B * S                    # 4096 flat tokens, batch-major
TT = 512                     # token tile (phase 1 / q tiles)
KT = 128                     # k tile (scores partition dim)
NT = T // TT                 # 8 token tiles
F32 = mybir.dt.float32
F32R = mybir.dt.float32r
AF = mybir.ActivationFunctionType
SCALE = 1.0 / np.sqrt(DK)

_cache = {}


def _consts():
    inv_freq = 10000.0 ** (-(np.arange(0, DK, 2, dtype=np.float64) / DK))
    pos = np.arange(S, dtype=np.float64)
    ang = pos[:, None] * inv_freq[None, :]                 # [S, 32]
    cos = np.repeat(np.cos(ang), 2, axis=1).T              # [64, S]
    sin = np.repeat(np.sin(ang), 2, axis=1).T
    cosT = np.concatenate([cos, cos], 0).astype(np.float32)   # [128, S]
    sinT = np.concatenate([sin, sin], 0).astype(np.float32)
    # P2T = P.T blockdiag for 2 heads; (P v)[2i] = -v[2i+1], (P v)[2i+1] = v[2i]
    p = np.zeros((DK, DK), np.float32)
    for i in range(DK // 2):
        p[2 * i, 2 * i + 1] = -1.0
        p[2 * i + 1, 2 * i] = 1.0
    p2t = np.zeros((128, 128), np.float32)
    p2t[:DK, :DK] = p.T
    p2t[DK:, DK:] = p.T
    ident = np.eye(128, dtype=np.float32)
    ones64 = np.ones((1, DK), np.float32)
    return cosT, sinT, p2t, ident, ones64


def split_multi_waits(nc, max_waits=1):
    """This walrus build allows fewer sync-waits per instruction than Tile's
    final drain carries; hoist extras onto same-engine NOPs inserted before."""
    for fn in nc.m.functions:
        for blk in fn.blocks:
            insts = blk.instructions
            out = []
            for inst in insts:
                si = getattr(inst, "sync_info", None)
                waits = list(si.on_wait) if si is not None else []
                if len(waits) > max_waits:
                    extra, keep = waits[:-max_waits], waits[-max_waits:]
                    for j, w in enumerate(extra):
                        nop = mybir.InstNoOp(
                            name=f"{inst.name}-wsplit{j}", ins=[], outs=[]
                        )
                        nop.engine = inst.engine
                        nop.sync_info = mybir.SyncInfo(on_wait=[w], on_update=[])
                        out.append(nop)
                    inst.sync_info = mybir.SyncInfo(
                        on_wait=keep, on_update=list(si.on_update)
                    )
                out.append(inst)
            insts[:] = out


def build_nc(repeat=1):
    cosT_np, sinT_np, p2t_np, ident_np, ones64_np = _consts()

    nc = bass.Bass("TRN2", target_bir_lowering=False, debug=False,
                   num_devices=N_CORES)
    xT = nc.declare_dram_parameter("xT", [D, T], F32R, isOutput=False)
    wq = nc.declare_dram_parameter("wq", [D, 128], F32R, isOutput=False)
    wk = nc.declare_dram_parameter("wk", [D, 128], F32R, isOutput=False)
    wv = nc.declare_dram_parameter("wv", [D, 128], F32R, isOutput=False)
    wo = nc.declare_dram_parameter("wo", [D, D], F32R, isOutput=False)
    y = nc.declare_dram_parameter("y", [TT, D], F32, isOutput=True)

    c_cos = nc.inline_tensor(cosT_np, name="c_cos")
    c_sin = nc.inline_tensor(sinT_np, name="c_sin")
    c_p2t = nc.inline_tensor(p2t_np, name="c_p2t")
    c_id = nc.inline_tensor(ident_np, name="c_id")
    c_on = nc.inline_tensor(ones64_np, name="c_on")

    a2a_ins = [nc.dram_tensor(f"a2a_in{r}", [N_CORES, 128, TT], F32R)
               for r in range(repeat)]
    a2a_outs = [nc.dram_tensor(f"a2a_out{r}", [N_CORES, 128, TT], F32R)
                for r in range(repeat)]

    with tile.TileContext(nc) as tc, ExitStack() as ctx:
        cst = ctx.enter_context(tc.tile_pool(name="cst", bufs=1))
        stream = ctx.enter_context(tc.tile_pool(name="stream", bufs=2))
        persist = ctx.enter_context(tc.tile_pool(name="persist", bufs=1))
        tmp = ctx.enter_context(tc.tile_pool(name="tmp", bufs=3))
        attnp = ctx.enter_context(tc.tile_pool(name="attnp", bufs=3))
        outp = ctx.enter_context(tc.tile_pool(name="outp", bufs=3))
        ps = ctx.enter_context(tc.tile_pool(name="ps", bufs=2, space="PSUM"))
        psav = ctx.enter_context(tc.tile_pool(name="psav", bufs=2, space="PSUM"))

        # ---- constants + weights to SBUF ----
        cos_s = cst.tile([128, S], F32)
        sin_s = cst.tile([128, S], F32)
        p2t_s = cst.tile([128, 128], F32R)
        id_s = cst.tile([128, 128], F32R)
        on_s = cst.tile([1, DK], F32R)
        wq_s = cst.tile([128, 8, 128], F32R)
        wk_s = cst.tile([128, 8, 128], F32R)
        wv_s = cst.tile([128, 8, 128], F32R)
        # ordered by first use: q-proj, rope-q, k-proj, v, transpose, denom
        nc.gpsimd.dma_start(
            out=wq_s[:], in_=wq.ap().rearrange("(g p) m -> p g m", p=128))
        nc.gpsimd.dma_start(out=sin_s[:], in_=c_sin[:, :])
        nc.gpsimd.dma_start(out=p2t_s[:], in_=c_p2t.ap().bitcast(F32R))
        nc.gpsimd.dma_start(out=cos_s[:], in_=c_cos[:, :])
        nc.gpsimd.dma_start(
            out=wk_s[:], in_=wk.ap().rearrange("(g p) m -> p g m", p=128))
        nc.gpsimd.dma_start(
            out=wv_s[:], in_=wv.ap().rearrange("(g p) m -> p g m", p=128))
        nc.gpsimd.dma_start(out=id_s[:], in_=c_id.ap().bitcast(F32R))
        nc.gpsimd.dma_start(out=on_s[:], in_=c_on.ap().bitcast(F32R))

        # persistent activations
        qrot = persist.tile([128, T], F32R)
        krot = persist.tile([128, T], F32R)
        v_sb = persist.tile([128, T // KT, 130], F32R)   # [.., 0:64]+ones | [.., 65:129]+ones
        # cols 64 and 129 stay 1.0 (denominator ones); memset needs an f32 view
        nc.vector.memset(v_sb[:].rearrange("p a b -> p (a b)").bitcast(F32), 1.0)

        cur = {}

        def phase1(t):
            """Project token tile t, rope q/k, transpose v."""
            xt = stream.tile([128, 8, TT], F32R, tag="xt")
            for g in range(8):
                nc.sync.dma_start(
                    out=xt[:, g, :],
                    in_=xT[g * 128:(g + 1) * 128, t * TT:(t + 1) * TT],
                )
            pos = slice((t % (S // TT)) * TT, (t % (S // TT)) * TT + TT)
            tok = slice(t * TT, (t + 1) * TT)
            for w_sb, dst in ((wq_s, qrot), (wk_s, krot)):
                p_q = ps.tile([128, TT], F32, tag="p1")
                for g in range(8):
                    nc.tensor.matmul(p_q[:], w_sb[:, g, :], xt[:, g, :],
                                     start=(g == 0), stop=(g == 7))
                qs = tmp.tile([128, TT], F32R, tag="qs")
                nc.vector.tensor_mul(qs[:], p_q[:], sin_s[:, pos])
                p_perm = ps.tile([128, TT], F32, tag="p1")
                nc.tensor.matmul(p_perm[:], p2t_s[:], qs[:], start=True, stop=True)
                qc = tmp.tile([128, TT], F32, tag="qc")
                nc.vector.tensor_mul(qc[:], p_q[:], cos_s[:, pos])
                nc.vector.tensor_add(dst[:, tok], qc[:], p_perm[:])
            # v: project then transpose to natural layout
            p_v = ps.tile([128, TT], F32, tag="p1")
            for g in range(8):
                nc.tensor.matmul(p_v[:], wv_s[:, g, :], xt[:, g, :],
                                 start=(g == 0), stop=(g == 7))
            vt = tmp.tile([128, TT], F32R, tag="vt")
            nc.vector.tensor_copy(vt[:], p_v[:])
            for blk in range(TT // 128):
                p_t = ps.tile([128, 128], F32R, tag="p1")
                nc.tensor.transpose(p_t[:], vt[:, blk * 128:(blk + 1) * 128], id_s[:])
                g = t * (TT // 128) + blk
                vdst = v_sb[:, g, :].rearrange("p (a c) -> p a c", a=2)[:, :, 0:64]
                nc.vector.tensor_copy(
                    vdst, p_t[:].rearrange("p (a c) -> p a c", a=2)
                )

        def attention(b, J):
            """q-tile J (512 wide) of batch b, both heads paired."""
            av0 = psav.tile([65, TT], F32, tag="av")
            av1 = psav.tile([65, TT], F32, tag="av")
            av = [av0, av1]
            nk = 4 * (J + 1)
            for i in range(nk):
                r = i - 4 * J          # >= 0 on diagonal blocks
                qo = KT * r if r > 0 else 0    # causal-narrowed q offset
                n = TT - qo
                p_s = ps.tile([128, 2, TT], F32, tag="mm")
                for h in range(2):
                    hp = slice(64 * h, 64 * h + 64)
                    nc.tensor.matmul(
                        p_s[:, h, 0:n],
                        krot[hp, b * S + i * KT: b * S + (i + 1) * KT],
                        qrot[hp, b * S + J * TT + qo: b * S + (J + 1) * TT],
                        start=True, stop=True,
                    )
                at = attnp.tile([128, 2, TT], F32R, tag="at")
                nc.scalar.activation(at[:, :, 0:n], p_s[:, :, 0:n], AF.Exp,
                                     scale=float(SCALE))
                if r >= 0:  # diagonal 128-block: zero where k > q
                    for h in range(2):
                        nc.gpsimd.affine_select(
                            out=at[:, h, 0:KT], in_=at[:, h, 0:KT],
                            compare_op=mybir.AluOpType.is_ge,
                            fill=0.0, base=0,
                            pattern=[[1, KT]], channel_multiplier=-1,
                        )
                g = (b * S) // KT + i
                for h in range(2):
                    nc.tensor.matmul(
                        av[h][:, qo:TT], v_sb[:, g, 65 * h:65 * h + 65],
                        at[:, h, 0:n],
                        start=(i == 0), stop=(i == nk - 1),
                    )
            # normalize by denominator row and stage for A2A
            stage = outp.tile([128, TT], F32R, tag="stage")
            for h in range(2):
                rec = tmp.tile([1, TT], F32R, tag="rec")
                with nc.allow_low_precision(reason="f32r recip for bcast matmul"):
                    nc.vector.reciprocal(rec[:], av[h][64:65, :])
                p_bc = ps.tile([64, TT], F32, tag="p1")
                nc.tensor.matmul(p_bc[:], on_s[:], rec[:], start=True, stop=True)
                hp = slice(64 * h, 64 * h + 64)
                nc.vector.tensor_copy(stage[hp, :], av[h][0:64, :])
                nc.vector.tensor_mul(stage[hp, :], stage[hp, :], p_bc[:])
            dest = 4 * b + J
            nc.sync.dma_start(out=cur["a2a_in"][dest, :, :], in_=stage[:])

        # interleave: attention(b, J) depends only on token tiles <= t
        wo_s0 = cst.tile([128, 8, TT], F32R)
        wo_s1 = cst.tile([128, 8, TT], F32R)
        for rep in range(repeat):
            cur["a2a_in"] = a2a_ins[rep]
            for t in range(8):
                phase1(t)
                attention(t // 4, t % 4)
                if rep == 0 and t == 0:  # preload Wo while DMA is idle
                    for n, w_sb in ((0, wo_s0), (1, wo_s1)):
                        nc.gpsimd.dma_start(
                            out=w_sb[:],
                            in_=wo[:, n * TT:(n + 1) * TT].rearrange(
                                "(g p) n -> p g n", p=128),
                        )

            nc.gpsimd.collective_compute(
                "AllToAll", mybir.AluOpType.bypass,
                replica_groups=[list(range(N_CORES))],
                ins=[a2a_ins[rep].ap().opt()], outs=[a2a_outs[rep].ap().opt()],
            )

            # ---- output projection on my 512-token row shard ----
            cat = persist.tile([128, 8, TT], F32R, tag="cat")
            for g in range(8):
                nc.sync.dma_start(out=cat[:, g, :], in_=a2a_outs[rep][g, :, :])
            for n, wo_s in ((0, wo_s0), (1, wo_s1)):
                for m in range(4):
                    po = ps.tile([128, TT], F32, tag="p1")
                    for g in range(8):
                        nc.tensor.matmul(po[:], cat[:, g, m * 128:(m + 1) * 128],
                                         wo_s[:, g, :], start=(g == 0), stop=(g == 7))
                    yt = outp.tile([128, TT], F32, tag="yt")
                    nc.vector.tensor_copy(yt[:], po[:])
                    nc.sync.dma_start(
                        out=y[m * 128:(m + 1) * 128, n * TT:(n + 1) * TT], in_=yt[:]
                    )

    split_multi_waits(nc)
    return nc


def _get_runner(repeat=1):
    """Build + jit once; returns f(in_maps) -> list of per-core output dicts."""
    key = ("runner", repeat)
    if key in _cache:
        return _cache[key]
    import jax
    import jax.numpy as jnp
    from jax.sharding import Mesh, PartitionSpec
    from jax.experimental.shard_map import shard_map
    from concourse import bass2jax, mybir as _mybir

    nc = build_nc(repeat=repeat)
    bass2jax.install_neuronx_cc_hook()

    in_names, out_names, out_avals, zero_outs = [], [], [], []
    for alloc in nc.m.functions[0].allocations:
        if not isinstance(_mybir.MemoryLocationSet, type) or not isinstance(
            alloc, _mybir.MemoryLocationSet
        ):
            continue
        name = alloc.memorylocations[0].name
        if alloc.kind == "ExternalInput":
            if name != "partition_id":
                in_names.append(name)
        elif alloc.kind == "ExternalOutput":
            out_names.append(name)
            shape = tuple(alloc.tensor_shape)
            dtype = _mybir.dt.np(alloc.dtype)
            out_avals.append(jax.core.ShapedArray(shape, dtype))
            zero_outs.append(np.zeros(shape, dtype))
    n_params = len(in_names)
    has_pid = nc.partition_id_tensor is not None
    all_names = in_names + out_names + (["partition_id"] if has_pid else [])

    def _body(*args):
        operands = list(args)
        if has_pid:
            operands.append(bass2jax.partition_id_tensor())
        outs = bass2jax._bass_exec_p.bind(
            *operands,
            out_avals=tuple(out_avals),
            in_names=tuple(all_names),
            out_names=tuple(out_names),
            lowering_input_output_aliases=(),
            sim_require_finite=True,
            sim_require_nnan=True,
            nc=nc,
        )
        return tuple(outs)

    devices = jax.devices()[:N_CORES]
    mesh = Mesh(np.asarray(devices), ("core",))
    n_outs = len(out_names)
    sharded = jax.jit(
        shard_map(
            _body, mesh=mesh,
            in_specs=(PartitionSpec("core"),) * (n_params + n_outs),
            out_specs=(PartitionSpec("core"),) * n_outs,
            check_rep=False,
        ),
        donate_argnums=tuple(range(n_params, n_params + n_outs)),
        keep_unused=True,
    )

    def make_bench(in_maps):
        from jax.sharding import NamedSharding
        sh = NamedSharding(mesh, PartitionSpec("core"))
        concat_in = [
            jax.device_put(
                np.concatenate([np.asarray(m[nm]) for m in in_maps], axis=0), sh)
            for nm in in_names
        ]
        zshapes = [(N_CORES * z.shape[0], *z.shape[1:]) for z in zero_outs]
        zdt = [z.dtype for z in zero_outs]
        mkz = jax.jit(
            lambda: tuple(jnp.zeros(s, d) for s, d in zip(zshapes, zdt)),
            out_shardings=tuple(sh for _ in zshapes),
        )

        def bench_once():
            zs = mkz()
            jax.block_until_ready(zs)
            t0 = __import__("time").perf_counter()
            out = sharded(*concat_in, *zs)
            jax.block_until_ready(out)
            return __import__("time").perf_counter() - t0

        return bench_once

    run_ns = {"make_bench": make_bench}

    def run(in_maps):
        concat_in = [
            np.concatenate([np.asarray(m[nm]) for m in in_maps], axis=0)
            for nm in in_names
        ]
        concat_zeros = [
            np.zeros((N_CORES * z.shape[0], *z.shape[1:]), z.dtype)
            for z in zero_outs
        ]
        out_arrs = sharded(*concat_in, *concat_zeros)
        return [
            {nm: np.asarray(out_arrs[i]).reshape(N_CORES, *out_avals[i].shape)[c]
             for i, nm in enumerate(out_names)}
            for c in range(N_CORES)
        ]

    run.make_bench = make_bench
    _cache[key] = run
    return run


def _prep_in_maps(x, Wq, Wk, Wv, Wo):
    xT = np.ascontiguousarray(x.reshape(T, D).T)
    wo = np.ascontiguousarray(Wo)
    in_maps = []
    for c in range(N_CORES):
        in_maps.append({
            "xT": xT,
            "wq": np.ascontiguousarray(np.concatenate([Wq[2 * c], Wq[2 * c + 1]], 1)),
            "wk": np.ascontiguousarray(np.concatenate([Wk[2 * c], Wk[2 * c + 1]], 1)),
            "wv": np.ascontiguousarray(np.concatenate([Wv[2 * c], Wv[2 * c + 1]], 1)),
            "wo": wo,
        })
    return in_maps


def kernel(x, Wq, Wk, Wv, Wo, repeat=1):
    x, Wq, Wk, Wv, Wo = (np.asarray(a, np.float32) for a in (x, Wq, Wk, Wv, Wo))
    run = _get_runner(repeat=repeat)
    results = run(_prep_in_maps(x, Wq, Wk, Wv, Wo))
    out = np.concatenate([r["y"] for r in results], axis=0)
    return out.reshape(B, S, D)
